# revision 12
# baseline (speedup 1.0000x reference)
"""Trainium2 Bass kernel for nn_MinibatchDiscrimination.

Reference math (f32):
    M = einsum('bi,ijk->bjk', x, T)                     # [512, 64, 16]
    L1[i,j,o] = sum_k |M[i,o,k] - M[j,o,k]|             # [512, 512, 64]
    c = exp(-L1) * (1 - eye)                            # mask self-pairs
    o_b = 0.5 * c.mean(axis=1)                          # [512, 64]
    out = concat([x, o_b], axis=1)                      # [512, 320]

Sharding: the i-index of the pairwise computation is split across 8 cores
(64 rows each). The program is SPMD-uniform: each core receives x ROTATED by
-64*c rows so that its own slab lands at pair-columns j'=0..63; only input
DATA differs between cores, never addresses.

Per-core device pipeline:
  1. x^T via PE transposes; M^T built directly in the layout
     MT4[(s,o)partition, (u, j')free] = M[j', o, 2u+s] via PE matmuls with a
     column-permuted T as the stationary operand  (k = 2u+s, u=0..7, s=0..1).
  2. For each slab row il (64 of them):
       - D_u = |MT4[:, u, :] - MT4[:, u, il]| fused in ONE op:
         DVE tensor_scalar(sub, abs_max 0) for most u, ACT Abs(scale=-1,
         bias=col) for the rest (engine balance).
       - k-contraction over the 128 (s,o) partitions via an indicator matmul
         on PE, accumulating L1 in PSUM [64o, 512j']; the self-pair column is
         masked by a rank-1 (ones x BIG*e_il) accumulate matmul.
       - ONE ACT Exp(scale=-1) with accum_out => A[:, il] = sum_j exp(-L1).
  3. A^T via PE, scale by 0.5/512, DMA out; x-slab passthrough.
"""

import numpy as np
from contextlib import ExitStack

import concourse.bass as bass
import concourse.tile as tile
from concourse import bacc, mybir
from concourse.bass_utils import run_bass_kernel_spmd

F32 = mybir.dt.float32

B = 512          # batch
INF = 256        # in_features
OUTF = 64        # out_features
KD = 16          # kernel dims
N_CORES = 8
SLAB = B // N_CORES          # 64 rows of i per core
NU = KD // 2                 # 8 u-chunks (pairs of k)
BIG = 30000.0                # L1 += BIG on the diagonal => exp -> 0
OSCALE = 0.5 / B             # exact power of two (2^-10)

# u-chunks computed on ACT (rest on DVE); tune for engine balance.
ACT_US = (0, 1, 2, 3)
ABS_MASK = 0x7FFFFFFF  # clear fp32 sign bit


def _build_nc():
    nc = bacc.Bacc("TRN2", target_bir_lowering=False, debug=False)

    x_d = nc.dram_tensor("x", [B, INF], F32, kind="ExternalInput").ap()
    # T host-permuted to [i, (u, s, o)]: column u*128 + s*64 + o = T[i, o, 2u+s],
    # so each u-chunk is a contiguous single-free-dim stationary operand.
    t_d = nc.dram_tensor("Tp", [INF, OUTF * KD], F32, kind="ExternalInput").ap()
    # BIG * I64 rows, flattened on one partition: row il = dmI[0, il*64:(il+1)*64]
    dm_d = nc.dram_tensor("dmask", [1, SLAB * SLAB], F32, kind="ExternalInput").ap()
    # indicator[(s,o), o'] = (o' == o): contracts the two s-halves per o
    ind_d = nc.dram_tensor("ind", [128, OUTF], F32, kind="ExternalInput").ap()
    id_d = nc.dram_tensor("ident", [128, 128], F32, kind="ExternalInput").ap()
    out_d = nc.dram_tensor("out", [SLAB, INF + OUTF], F32, kind="ExternalOutput").ap()

    with tile.TileContext(nc) as tc, ExitStack() as ctx:
        consts = ctx.enter_context(tc.tile_pool(name="consts", bufs=1))
        work = ctx.enter_context(tc.tile_pool(name="work", bufs=1))
        dpool = ctx.enter_context(tc.tile_pool(name="dpool", bufs=8))
        epool = ctx.enter_context(tc.tile_pool(name="epool", bufs=2))
        ps_l1 = ctx.enter_context(tc.tile_pool(name="ps_l1", bufs=2, space="PSUM"))
        ps_fin = ctx.enter_context(tc.tile_pool(name="ps_fin", bufs=1, space="PSUM"))

        # ---- constants ----
        dm_sb = consts.tile([1, SLAB * SLAB], F32, tag="dm", name="dm_sb")
        nc.sync.dma_start(dm_sb, dm_d)
        ind_sb = consts.tile([128, OUTF], F32, tag="ind", name="ind_sb")
        nc.sync.dma_start(ind_sb, ind_d)
        id_sb = consts.tile([128, 128], F32, tag="ident", name="id_sb")
        nc.sync.dma_start(id_sb, id_d)
        ones_sb = consts.tile([1, OUTF], F32, tag="ones", name="ones_sb")
        nc.vector.memset(ones_sb, 1.0)

        MT4 = consts.tile([128, NU * B], F32, tag="mt4", name="MT4")
        A = work.tile([OUTF, SLAB], F32, tag="A", name="A")

        # ---- production: x^T, then MT4 ----
        with tc.tile_pool(name="prod", bufs=1) as prod, \
             tc.tile_pool(name="ps_prod", bufs=2, space="PSUM") as ps_prod:
            x_sb = []
            for bc in range(4):
                xt_ = prod.tile([128, INF], F32, tag=f"xsb{bc}", name=f"x_sb{bc}")
                nc.sync.dma_start(xt_, x_d[bc * 128:(bc + 1) * 128, :])
                x_sb.append(xt_)
            # x slab passthrough (rows 0..63 of rotated x = this core's slab)
            nc.sync.dma_start(out_d[:, 0:INF], x_sb[0][0:SLAB, :])

            t_sb = []
            for ic in range(2):
                tt_ = prod.tile([128, OUTF * KD], F32, tag=f"tsb{ic}", name=f"t_sb{ic}")
                nc.sync.dma_start(tt_, t_d[ic * 128:(ic + 1) * 128, :])
                t_sb.append(tt_)

            xT = []
            for ic in range(2):
                ps = ps_prod.tile([128, B], F32, tag="pst", name=f"ps_xt{ic}")
                for bc in range(4):
                    nc.tensor.transpose(
                        ps[:, bc * 128:(bc + 1) * 128],
                        x_sb[bc][:, ic * 128:(ic + 1) * 128],
                        id_sb,
                    )
                xt2_ = prod.tile([128, B], F32, tag=f"xT{ic}", name=f"xT{ic}")
                if ic == 0:
                    nc.scalar.copy(xt2_, ps)
                else:
                    nc.vector.tensor_copy(xt2_, ps)
                xT.append(xt2_)

            # MT4[:, u*B:(u+1)*B][p=(s,o), j] = M[j, o, 2u+s]
            # lhsT = T columns permuted to (s, o) order: T[p, o, 2u+s]
            for u in range(NU):
                ps = ps_prod.tile([128, B], F32, tag="pst", name=f"ps_mt{u}")
                for ic in range(2):
                    lhs = t_sb[ic][:, u * 128:(u + 1) * 128]
                    nc.tensor.matmul(ps, lhs, xT[ic], start=(ic == 0), stop=(ic == 1))
                if u % 2 == 0:
                    nc.scalar.copy(MT4[:, u * B:(u + 1) * B], ps)
                else:
                    nc.vector.tensor_copy(MT4[:, u * B:(u + 1) * B], ps)

        # ---- main loop over slab rows ----
        for il in range(SLAB):
            L1 = ps_l1.tile([OUTF, B], F32, tag="L1", name=f"L1_{il}")
            for u in range(NU):
                D = dpool.tile([128, B], F32, tag="D", name=f"D_{il}_{u}")
                col = MT4[:, u * B + il: u * B + il + 1]
                if u in ACT_US:
                    # |M_col - in| = Abs(-in + bias)
                    nc.scalar.activation(
                        D, MT4[:, u * B:(u + 1) * B],
                        mybir.ActivationFunctionType.Abs,
                        bias=col, scale=-1.0,
                    )
                else:
                    # d = in - col, then |d| by clearing the sign bit
                    nc.vector.tensor_scalar(
                        D, MT4[:, u * B:(u + 1) * B], col, None,
                        mybir.AluOpType.subtract,
                    )
                    du = D.bitcast(mybir.dt.uint32)
                    nc.vector.tensor_scalar(
                        du, du, ABS_MASK, None, mybir.AluOpType.bitwise_and,
                    )
                nc.tensor.matmul(L1, ind_sb, D, start=(u == 0), stop=(u == NU - 1))
                if u == 0:
                    # diagonal mask: L1[:, il] += 2*BIG (rank-1 accumulate)
                    nc.tensor.matmul(
                        L1[:, 0:SLAB], ones_sb,
                        dm_sb[:, il * SLAB:(il + 1) * SLAB],
                        start=False, stop=False,
                    )
            E = epool.tile([OUTF, B], F32, tag="E", name=f"E_{il}")
            nc.scalar.activation(
                E, L1, mybir.ActivationFunctionType.Exp,
                scale=-1.0, accum_out=A[:, il:il + 1],
            )

        # ---- finale: o_b = A^T * 0.5/B ----
        psA = ps_fin.tile([SLAB, OUTF], F32, tag="psA", name="psA")
        nc.tensor.transpose(psA, A, id_sb[0:SLAB, 0:SLAB])
        ob = work.tile([SLAB, OUTF], F32, tag="ob", name="ob")
        nc.scalar.mul(ob, psA, OSCALE)
        nc.sync.dma_start(out_d[:, INF:], ob)

    nc.compile()
    return nc


_NC = None


def _get_nc():
    global _NC
    if _NC is None:
        _NC = _build_nc()
    return _NC


def _host_inputs(x, T):
    ind = np.zeros((128, OUTF), np.float32)
    ind[np.arange(128), np.arange(128) % OUTF] = 1.0
    ident = np.eye(128, dtype=np.float32)
    dm = (BIG * np.eye(SLAB, dtype=np.float32)).reshape(1, -1)
    # [i, o, (u s)] -> [i, (u s o)]
    Tp = np.ascontiguousarray(
        T.reshape(INF, OUTF, NU, 2).transpose(0, 2, 3, 1).reshape(INF, OUTF * KD)
    )
    in_maps = []
    for c in range(N_CORES):
        xr = np.ascontiguousarray(np.roll(x, -c * SLAB, axis=0))
        in_maps.append({"x": xr, "Tp": Tp, "dmask": dm, "ind": ind, "ident": ident})
    return in_maps


def _run(x, T, trace=False):
    x = np.ascontiguousarray(np.asarray(x, dtype=np.float32))
    T = np.ascontiguousarray(np.asarray(T, dtype=np.float32))
    assert x.shape == (B, INF) and T.shape == (INF, OUTF, KD)
    nc = _get_nc()
    in_maps = _host_inputs(x, T)
    res = run_bass_kernel_spmd(nc, in_maps, list(range(N_CORES)), trace=trace)
    out = np.concatenate([res.results[c]["out"] for c in range(N_CORES)], axis=0)
    return out, res


def kernel(x, T):
    out, _ = _run(x, T, trace=False)
    return out


def kernel_profiled(x, T):
    out, res = _run(x, T, trace=True)
    return out, res


# revision 25
# speedup vs baseline: 413.8613x; 413.8613x over previous
"""Trainium2 Bass kernel for nn_MinibatchDiscrimination.

Reference math (f32):
    M = einsum('bi,ijk->bjk', x, T)                     # [512, 64, 16]
    L1[i,j,o] = sum_k |M[i,o,k] - M[j,o,k]|             # [512, 512, 64]
    c = exp(-L1) * (1 - eye)                            # mask self-pairs
    o_b = 0.5 * c.mean(axis=1)                          # [512, 64]
    out = concat([x, o_b], axis=1)                      # [512, 320]

Sharding: the i-index of the pairwise computation is split across 8 cores
(64 rows each). The program is SPMD-uniform: each core receives x ROTATED by
-64*c rows so that its own slab lands at pair-columns j'=0..63; only input
DATA differs between cores, never addresses.

Symmetry: c[i,j]=c[j,i], so each row il only processes the 256-wide window
j' in [il+1, il+256]. Every unordered pair {a,b} with offset d=(b-a) mod 512:
row a's window covers d in [1,256] (-> A[a] via ACT accum_out), and the
column-partial C (built from window columns 1..255 only) covers the partner
side; together each pair contributes to A[a] and A[b] exactly once, with the
self-pair excluded by construction. The host combines row-sums + scattered
column-partials from all cores.

Per-core device pipeline:
  1. x^T via PE transposes; M^T built in the layout
     MT4[(s,o)partition, (u, j')free] = M[j', o, 2u+s] via PE matmuls with a
     column-permuted T as the stationary operand (k = 2u+s). Only j' < 384
     columns are ever read, so production stops there.
  2. For each slab row il:
       - D_u = |MT4[:, u, win] - MT4[:, u, il]| on DVE (tensor_scalar
         subtract, then sign-bit clear on a uint32 view) and ACT (Abs
         activation with bias, scale=-1), split for engine balance;
       - k-contraction over the 128 (s,o) partitions via an indicator matmul
         on PE in f32r (single-pass fp32), accumulating L1 in PSUM [64o, 256];
       - ONE ACT Exp(scale=-1) with accum_out -> A[:, il] = row-sum; the exp
         tile E also feeds C[:, il+1:il+256] += E[:, :255] on GPSIMD.
  3. DMA out A [64o, 64il] and C [64o, 320]; host does the final scatter,
     scale by 0.5/512, and concat with x.
"""

import numpy as np
from contextlib import ExitStack

import concourse.bass as bass
import concourse.tile as tile
from concourse import bacc, dve_ops, mybir
from concourse.bass_utils import run_bass_kernel_spmd
from concourse.dve_spec import Spec, Src0, C0, maxx

F32 = mybir.dt.float32

# Fused |in0 - s0| as a single custom DVE instruction (one pass instead of
# subtract + separate abs). Registered at import time; the per-NEFF DVE table
# is generated from this spec automatically.
if "TENSOR_ABSDIFF_ANT" not in dve_ops._SUB_OPCODE_FOR_NAME:
    ABSDIFF = dve_ops.DveOp(
        "TENSOR_ABSDIFF_ANT",
        Spec(
            body=maxx(Src0 - C0, C0 - Src0),
            reference=lambda in0, in1, s0, s1, imm2:
                np.abs(in0 - s0).astype(np.float32),
        ),
        subdim=False,
        uops_sha={"v3": "a5866c869c7d6e30", "v4": "006fe4b232e6035a"},
    )
    dve_ops.OPS.append(ABSDIFF)
    dve_ops.CUSTOM_DVE_SPECS[ABSDIFF.name] = ABSDIFF.spec
    dve_ops._SUB_OPCODE_FOR_NAME[ABSDIFF.name] = (
        max(dve_ops._SUB_OPCODE_FOR_NAME.values()) + 1
    )
else:  # re-import safety
    ABSDIFF = next(op for op in dve_ops.OPS if op.name == "TENSOR_ABSDIFF_ANT")

B = 512          # batch
INF = 256        # in_features
OUTF = 64        # out_features
KD = 16          # kernel dims
N_CORES = 8
SLAB = B // N_CORES          # 64 rows of i per core
NU = KD // 2                 # 8 u-chunks (pairs of k)
W = 256                      # symmetric window width
MTW = 384                    # produced MT4 width (cols ever read: < 320)
OSCALE = 0.5 / B             # exact power of two (2^-10)
ABS_MASK = 0x7FFFFFFF        # clear fp32 sign bit


def _build_nc(n_rows=SLAB):
    nc = bacc.Bacc("TRN2", target_bir_lowering=False, debug=False)

    x_d = nc.dram_tensor("x", [B, INF], F32, kind="ExternalInput").ap()
    # T host-permuted to [i, (u, s, o)]: column u*128 + s*64 + o = T[i, o, 2u+s],
    # so each u-chunk is a contiguous single-free-dim stationary operand.
    t_d = nc.dram_tensor("Tp", [INF, OUTF * KD], F32, kind="ExternalInput").ap()
    # indicator[(s,o), o'] = (o' == o): contracts the two s-halves per o.
    # Declared f32r end-to-end so the f32r k-contraction matmuls verify.
    ind_d = nc.dram_tensor("ind", [128, OUTF], mybir.dt.float32r,
                           kind="ExternalInput").ap()
    id_d = nc.dram_tensor("ident", [128, 128], F32, kind="ExternalInput").ap()
    a_d = nc.dram_tensor("outa", [OUTF, SLAB], F32, kind="ExternalOutput").ap()
    c_d = nc.dram_tensor("outc", [OUTF, 320], F32, kind="ExternalOutput").ap()

    with tile.TileContext(nc) as tc, ExitStack() as ctx:
        consts = ctx.enter_context(tc.tile_pool(name="consts", bufs=1))
        work = ctx.enter_context(tc.tile_pool(name="work", bufs=1))
        dpool = ctx.enter_context(tc.tile_pool(name="dpool", bufs=12))
        epool = ctx.enter_context(tc.tile_pool(name="epool", bufs=4))
        ps_l1 = ctx.enter_context(tc.tile_pool(name="ps_l1", bufs=4, space="PSUM"))

        # ---- constants ----
        ind_sb = consts.tile([128, OUTF], mybir.dt.float32r, tag="ind", name="ind_sb")
        nc.sync.dma_start(ind_sb, ind_d)
        id_sb = consts.tile([128, 128], F32, tag="ident", name="id_sb")
        nc.sync.dma_start(id_sb, id_d)

        MT4 = consts.tile([128, NU * MTW], F32, tag="mt4", name="MT4")
        A = work.tile([OUTF, SLAB], F32, tag="A", name="A")
        C = work.tile([OUTF, 320], F32, tag="C", name="C")
        nc.gpsimd.memset(C, 0.0)

        # ---- production: x^T (384 rows), then MT4 ----
        with tc.tile_pool(name="prod", bufs=1) as prod, \
             tc.tile_pool(name="ps_prod", bufs=2, space="PSUM") as ps_prod:
            x_sb = []
            for bc in range(3):
                xt_ = prod.tile([128, INF], F32, tag=f"xsb{bc}", name=f"x_sb{bc}")
                nc.sync.dma_start(xt_, x_d[bc * 128:(bc + 1) * 128, :])
                x_sb.append(xt_)

            t_sb = []
            for ic in range(2):
                tt_ = prod.tile([128, OUTF * KD], F32, tag=f"tsb{ic}", name=f"t_sb{ic}")
                nc.sync.dma_start(tt_, t_d[ic * 128:(ic + 1) * 128, :])
                t_sb.append(tt_)

            xT = []
            for ic in range(2):
                ps = ps_prod.tile([128, MTW], F32, tag="pst", name=f"ps_xt{ic}")
                for bc in range(3):
                    nc.tensor.transpose(
                        ps[:, bc * 128:(bc + 1) * 128],
                        x_sb[bc][:, ic * 128:(ic + 1) * 128],
                        id_sb,
                    )
                xt2_ = prod.tile([128, MTW], F32, tag=f"xT{ic}", name=f"xT{ic}")
                if ic == 0:
                    nc.scalar.copy(xt2_, ps)
                else:
                    nc.vector.tensor_copy(xt2_, ps)
                xT.append(xt2_)

            # MT4[:, u*MTW:(u+1)*MTW][p=(s,o), j] = M[j, o, 2u+s]
            for u in range(NU):
                ps = ps_prod.tile([128, MTW], F32, tag="pst", name=f"ps_mt{u}")
                for ic in range(2):
                    lhs = t_sb[ic][:, u * 128:(u + 1) * 128]
                    nc.tensor.matmul(ps, lhs, xT[ic], start=(ic == 0), stop=(ic == 1))
                if u % 2 == 0:
                    nc.scalar.copy(MT4[:, u * MTW:(u + 1) * MTW], ps)
                else:
                    nc.vector.tensor_copy(MT4[:, u * MTW:(u + 1) * MTW], ps)

        # ---- main loop over slab rows ----
        F32R = mybir.dt.float32r
        for il in range(n_rows):
            n_act = 3  # u-chunks on ACT; rest on DVE (fused custom absdiff)
            L1 = ps_l1.tile([OUTF, W], F32, tag="L1", name=f"L1_{il}")
            for u in range(NU):
                D = dpool.tile([128, W], F32R, tag="D", name=f"D_{il}_{u}")
                src = MT4[:, u * MTW + il + 1: u * MTW + il + 1 + W]
                col = MT4[:, u * MTW + il: u * MTW + il + 1]
                if u < n_act:
                    # |col - in| = Abs(-in + bias)
                    nc.scalar.activation(
                        D, src, mybir.ActivationFunctionType.Abs,
                        bias=col, scale=-1.0,
                    )
                else:
                    nc.vector._custom_dve(ABSDIFF, out=D, in0=src, s0=col)
                # k-contraction in f32r (single-pass fp32): weights are exact
                # 1s, only D is rounded — error vanishes under exp at this scale
                nc.tensor.matmul(L1, ind_sb, D,
                                 start=(u == 0), stop=(u == NU - 1))
            E = epool.tile([OUTF, W], F32, tag="E", name=f"E_{il}")
            nc.scalar.activation(
                E, L1, mybir.ActivationFunctionType.Exp,
                scale=-1.0, accum_out=A[:, il:il + 1],
            )
            # column partials: C[:, il+1 : il+256] += E[:, :255]
            nc.gpsimd.tensor_add(
                C[:, il + 1: il + W], C[:, il + 1: il + W], E[:, 0:W - 1],
            )

        nc.sync.dma_start(a_d, A)
        nc.sync.dma_start(c_d, C)

    nc.compile()
    return nc


_NC = None


def _get_nc():
    global _NC
    if _NC is None:
        _NC = _build_nc()
    return _NC


def _host_inputs(x, T):
    ind = np.zeros((128, OUTF), np.float32)
    ind[np.arange(128), np.arange(128) % OUTF] = 1.0
    ident = np.eye(128, dtype=np.float32)
    # [i, o, (u s)] -> [i, (u s o)]
    Tp = np.ascontiguousarray(
        T.reshape(INF, OUTF, NU, 2).transpose(0, 2, 3, 1).reshape(INF, OUTF * KD)
    )
    in_maps = []
    for c in range(N_CORES):
        xr = np.ascontiguousarray(np.roll(x, -c * SLAB, axis=0))
        in_maps.append({"x": xr, "Tp": Tp, "ind": ind, "ident": ident})
    return in_maps


def _assemble(x, results):
    """Combine per-core row-sums and column-partials into the full output."""
    At = np.zeros((B, OUTF), np.float64)
    for c in range(N_CORES):
        a = np.asarray(results[c]["outa"])   # [OUTF, SLAB]
        cp = np.asarray(results[c]["outc"])  # [OUTF, 320]
        At[c * SLAB:(c + 1) * SLAB, :] += a.T
        jj = (np.arange(320) + c * SLAB) % B
        np.add.at(At, jj, cp.T.astype(np.float64))
    o_b = (At * OSCALE).astype(np.float32)
    return np.concatenate([x, o_b], axis=1)


def _run(x, T, trace=False):
    x = np.ascontiguousarray(np.asarray(x, dtype=np.float32))
    T = np.ascontiguousarray(np.asarray(T, dtype=np.float32))
    assert x.shape == (B, INF) and T.shape == (INF, OUTF, KD)
    nc = _get_nc()
    in_maps = _host_inputs(x, T)
    res = run_bass_kernel_spmd(nc, in_maps, list(range(N_CORES)), trace=trace)
    return _assemble(x, res.results), res


def kernel(x, T):
    out, _ = _run(x, T, trace=False)
    return out


def kernel_profiled(x, T):
    out, res = _run(x, T, trace=True)
    return out, res
